# revision 1
# baseline (speedup 1.0000x reference)
"""Trainium2 Bass kernel for a batched linear-chain CRF negative log-likelihood.

reference semantics (B=128, S=2048, T=128):
    forward algorithm over S steps (log-space matvec chain) -> log_Z per batch
    gold path score = emissions gathered at tags + transitions gathered at
    (tag_t, tag_{t+1}) pairs, summed over time
    output = mean(log_Z - seq_score)   (scalar f32)

Strategy (v2):
  - data parallel over 8 cores: 16 batch rows per core, transitions replicated.
  - linear space: a_t = (a_{t-1} @ W) * E_t with W = exp(transitions),
    E_t = exp(emit_t - chat).  Per-step work: one PE matmul (stationary W,
    moving state [128 tags x 16 batch]) + one DVE multiply out of PSUM.
  - bidirectional: forward chain from t=0 and a backward chain
    y_t = E_t * (W @ y_{t+1}) from t=2047 run concurrently and meet at
    t=1023: log_Z = log(a_m . (W y_{m+1})) + accumulated log scales.
  - renormalization every 64 steps; colsum scale logs parked and ln'd once
    in the epilogue.
  - E precompute reads a HOST-pretransposed copy of emissions
    ([b, tag, t] layout) so no PE transposes are needed: DMA chunk + one
    scalar-engine exp (bias = -chat) straight into the [tag, b*S+t] bf16
    buffer.  This keeps the PE free for the latency-critical chain.
  - gold path one-hot building and the emissions bf16 copy run on the
    otherwise-idle GPSIMD engine (they competed with the chain multiplies
    on DVE / scalar in v1); the count-matrix accumulation stays on the PE
    as two N=128 matmuls per (b, sblock); finalize per batch row is
    scalar-engine evacuation + GPSIMD multiply/reduce.
  - PSUM chain tiles are padded to a full 2KB bank to avoid read/write
    bank contention between the PE drain and the DVE multiply.
"""

import numpy as np

B, S, T = 128, 2048, 128
NCORES = 8
BC = B // NCORES  # 16 batch rows per core
NSB = S // 128  # 16 s-blocks of 128
MID = S // 2 - 1  # 1023: chains meet here
RENORM = 64
ECH = 256  # E-precompute chunk (columns of t per DMA+exp)
NCH = S // ECH
JUNK_TAG = 60000.0  # one-hot of this is all zeros (tags are < 128)

_compiled = None


def _build_program(use_gpsimd=True):
    import concourse.bass as bass
    import concourse.bacc as bacc
    import concourse.tile as tile
    from concourse import mybir
    from concourse.masks import make_identity

    fp32 = mybir.dt.float32
    bf16 = mybir.dt.bfloat16
    AF = mybir.ActivationFunctionType
    ALU = mybir.AluOpType
    AX = mybir.AxisListType

    nc = bacc.Bacc(None)
    em_d = nc.declare_dram_parameter("emissions_sh", [BC, S, T], fp32, isOutput=False)
    emt_d = nc.declare_dram_parameter("emis_t", [BC, T, S], fp32, isOutput=False)
    tr_d = nc.declare_dram_parameter("transitions", [T, T], fp32, isOutput=False)
    tg_d = nc.declare_dram_parameter("tags_sh", [BC, S], mybir.dt.int32, isOutput=False)
    out_d = nc.declare_dram_parameter("loss_parts", [BC], fp32, isOutput=True)

    gp = nc.gpsimd if use_gpsimd else nc.vector

    with tile.TileContext(nc) as tc:
        with (
            tc.tile_pool(name="consts", bufs=1) as consts,
            tc.tile_pool(name="ebuf", bufs=1) as ebufp,
            tc.tile_pool(name="et", bufs=4) as etp,
            tc.tile_pool(name="emis", bufs=12) as emisp,
            tc.tile_pool(name="oh", bufs=8) as ohp,
            tc.tile_pool(name="dump", bufs=4) as dumpp,
            tc.tile_pool(name="state", bufs=8) as statep,
            tc.tile_pool(name="small", bufs=6) as smallp,
            tc.tile_pool(name="tp_ps", bufs=1, space="PSUM") as tp_ps,
            tc.tile_pool(name="q_ps", bufs=4, space="PSUM") as q_ps,
            tc.tile_pool(name="cd_ps", bufs=1, space="PSUM") as cd_ps,
            tc.tile_pool(name="cd2_ps", bufs=1, space="PSUM") as cd2_ps,
            tc.tile_pool(name="m_ps", bufs=1, space="PSUM") as m_ps,
        ):
            # ---------------- constants ----------------
            ident = consts.tile([128, 128], fp32)
            make_identity(nc, ident)
            ident_bf = consts.tile([128, 128], bf16)
            make_identity(nc, ident_bf)
            iota = consts.tile([128, 128], bf16)
            nc.gpsimd.iota(
                iota, pattern=[[1, 128]], base=0, channel_multiplier=0,
                allow_small_or_imprecise_dtypes=True,
            )
            ones_col_bf = consts.tile([128, 1], bf16)
            nc.vector.memset(ones_col_bf, 1.0)
            ones_col_f = consts.tile([128, 1], fp32)
            nc.vector.memset(ones_col_f, 1.0)
            ones_row_f = consts.tile([1, 128], fp32)
            nc.vector.memset(ones_row_f, 1.0)

            # transitions -> W = exp(trans) bf16, WT = W^T bf16
            tr_sb = consts.tile([128, 128], fp32)
            nc.sync.dma_start(out=tr_sb, in_=tr_d[:, :])
            w_bf = consts.tile([128, 128], bf16)
            nc.scalar.activation(w_bf, tr_sb, AF.Exp)
            wt_psum = tp_ps.tile([128, 128], bf16, tag="tp")
            nc.tensor.transpose(wt_psum, w_bf, ident_bf)
            wt_bf = consts.tile([128, 128], bf16)
            nc.vector.tensor_copy(wt_bf, wt_psum)

            # [trans | identity] for the gold finalize
            tri = consts.tile([128, 256], fp32)
            nc.vector.tensor_copy(tri[:, 0:128], tr_sb)
            nc.vector.tensor_copy(tri[:, 128:256], ident)

            # chat = mean_j ln(colsum_j W) over j=1..127  (col 0 is exp(-1e4)=0)
            colw_ps = m_ps.tile([1, 128], fp32, tag="m")
            nc.tensor.matmul(colw_ps, ones_col_bf, w_bf, start=True, stop=True)
            lncol = smallp.tile([1, 127], fp32, tag="lncol")
            lnsum = consts.tile([1, 1], fp32)
            nc.scalar.activation(lncol, colw_ps[:, 1:128], AF.Ln, accum_out=lnsum)
            chat_tot = consts.tile([1, 1], fp32)
            nc.scalar.activation(chat_tot, lnsum, AF.Copy, scale=float(S) / 127.0)
            negchat = consts.tile([1, 1], fp32)
            nc.scalar.activation(negchat, lnsum, AF.Copy, scale=-1.0 / 127.0)
            nbc_ps = m_ps.tile([128, 1], fp32, tag="m")
            nc.tensor.matmul(nbc_ps, ones_row_f, negchat, start=True, stop=True)
            negchat_bc = consts.tile([128, 1], fp32)
            nc.vector.tensor_copy(negchat_bc, nbc_ps)

            # tags -> f32, transposed into [s(128), (sb,b)] column layout,
            # plus a shift-by-one variant for transition pairs
            tags_nat = consts.tile([BC, S], mybir.dt.int32)
            nc.sync.dma_start(out=tags_nat, in_=tg_d[:, :])
            tags_f = consts.tile([BC, S], fp32)
            nc.vector.tensor_copy(tags_f, tags_nat)
            tag_cols = consts.tile([128, NSB * BC], fp32)   # col = sb*16 + b
            tagsh_cols = consts.tile([128, NSB * BC], fp32)
            nc.vector.memset(tagsh_cols[:, (NSB - 1) * BC:], JUNK_TAG)
            for sb in range(NSB):
                tp = tp_ps.tile([128, BC], fp32, tag="tp")
                nc.tensor.transpose(
                    tp, tags_f[:, sb * 128:(sb + 1) * 128], ident[:BC, :BC]
                )
                nc.vector.tensor_copy(tag_cols[:, sb * BC:(sb + 1) * BC], tp)
            for sb in range(NSB):
                n = 128 if sb < NSB - 1 else 127
                tp = tp_ps.tile([128, BC], fp32, tag="tp")
                nc.tensor.transpose(
                    tp[:n], tags_f[:, sb * 128 + 1: sb * 128 + 1 + n],
                    ident[:BC, :BC],
                )
                nc.vector.tensor_copy(
                    tagsh_cols[:n, sb * BC:(sb + 1) * BC], tp[:n]
                )

            # ---------------- E precompute (no PE involvement) ----------------
            ebuf = ebufp.tile([128, S * BC], bf16)  # free index = b*S + t
            ebuf3 = ebuf.rearrange("p (b t) -> p b t", t=S)

            def emit_E(b, c):
                et = etp.tile([128, ECH], fp32, tag="et")
                nc.sync.dma_start(out=et, in_=emt_d[b, :, c * ECH:(c + 1) * ECH])
                nc.scalar.activation(
                    ebuf3[:, b, c * ECH:(c + 1) * ECH], et, AF.Exp,
                    bias=negchat_bc,
                )

            # fw consumes chunks 0..(NCH/2-1) ascending; bw consumes
            # NCH-1 .. NCH/2 descending.  Emit in deadline order.
            for k in range(NCH // 2):
                for b in range(BC):
                    emit_E(b, k)
                for b in range(BC):
                    emit_E(b, NCH - 1 - k)

            # ---------------- gold side work ----------------
            # per-b [sum(C*trans) | esel] results: cols [2b, 2b+1]
            gsum = consts.tile([128, 2 * BC], fp32)
            gold_cd = [None]

            def gold_dma(b, sb):
                emis2 = emisp.tile([128, 128], fp32, tag="emis2",
                                   name=f"ge_{b}_{sb}")
                nc.sync.dma_start(
                    out=emis2, in_=em_d[b, sb * 128:(sb + 1) * 128, :]
                )
                gold_sub.dmas[(b, sb)] = emis2

            gold_cd2 = [None]

            def gold_sub(b, sb, phase):
                col = sb * BC + b
                eng1 = nc.vector
                eng2 = nc.vector
                if phase == 0:
                    # one-hot + shifted one-hot + emis bf16 (GPSIMD/DVE split)
                    emis2 = gold_sub.dmas.pop((b, sb))
                    oh = ohp.tile([128, 128], bf16, tag="oh", name=f"oh_{b}_{sb}")
                    eng1.tensor_scalar(
                        out=oh, in0=iota, scalar1=tag_cols[:, col:col + 1],
                        scalar2=None, op0=ALU.is_equal,
                    )
                    ohsh = ohp.tile([128, 128], bf16, tag="ohsh", name=f"os_{b}_{sb}")
                    eng2.tensor_scalar(
                        out=ohsh, in0=iota,
                        scalar1=tagsh_cols[:, col:col + 1],
                        scalar2=None, op0=ALU.is_equal,
                    )
                    emis_bf = ohp.tile([128, 128], bf16, tag="ebf", name=f"eb_{b}_{sb}")
                    gp.tensor_copy(emis_bf, emis2)
                    gold_sub.tiles[(b, sb)] = (oh, ohsh, emis_bf)
                elif phase == 1:
                    oh, ohsh, emis_bf = gold_sub.tiles[(b, sb)]
                    if sb == 0:
                        gold_cd[0] = cd_ps.tile(
                            [128, 512], fp32, tag="cd", name=f"gold_cd_{b}"
                        )
                        gold_cd2[0] = cd2_ps.tile(
                            [128, 512], fp32, tag="cd2", name=f"gold_cd2_{b}"
                        )
                    nc.tensor.matmul(
                        gold_cd[0][:, 0:128], oh, ohsh,
                        start=(sb == 0), stop=(sb == NSB - 1),
                    )
                else:
                    oh, ohsh, emis_bf = gold_sub.tiles.pop((b, sb))
                    nc.tensor.matmul(
                        gold_cd2[0][:, 0:128], oh, emis_bf,
                        start=(sb == 0), stop=(sb == NSB - 1),
                    )
                    if sb == NSB - 1:
                        # finalize row b: evacuate on ACT, [C|D]*[trans|ident]
                        # on GPSIMD, grouped reduce on DVE
                        cdump = dumpp.tile([128, 256], fp32, tag="cdump")
                        nc.scalar.activation(cdump[:, 0:128], gold_cd[0][:, 0:128], AF.Copy)
                        nc.scalar.activation(cdump[:, 128:256], gold_cd2[0][:, 0:128], AF.Copy)
                        cmul = dumpp.tile([128, 256], fp32, tag="cmul")
                        gp.tensor_tensor(out=cmul, in0=cdump, in1=tri, op=ALU.mult)
                        nc.vector.tensor_reduce(
                            gsum[:, 2 * b:2 * b + 2],
                            cmul.rearrange("p (c j) -> p c j", c=2),
                            axis=AX.X, op=ALU.add,
                        )

            gold_sub.tiles = {}
            gold_sub.dmas = {}

            units = []
            dma_side = []
            for b in range(BC):
                for sb in range(NSB):
                    dma_side.append((b, sb))
                    units.append((b, sb))
            side = []
            for u in range(len(units) + 2):
                if u < len(units):
                    side.append(units[u] + (0,))
                if u >= 2:
                    side.append(units[u - 2] + (1,))
                    side.append(units[u - 2] + (2,))

            def do_side(n):
                for _ in range(n):
                    # keep ~8 gold emis DMAs in flight ahead of use
                    while dma_side and len(gold_sub.dmas) < 8:
                        bd, sbd = dma_side.pop(0)
                        gold_dma(bd, sbd)
                    if side:
                        b, sb, ph = side.pop(0)
                        gold_sub(b, sb, ph)

            # ---------------- chain ----------------
            NRE = 64
            glog = consts.tile([1, BC * NRE], fp32)
            nc.vector.memset(glog, 1.0)
            glog3 = glog.rearrange("p (b k) -> p b k", k=NRE)
            renorm_k = [0]

            def renorm(v):
                """colsum -> reciprocal -> broadcast; park colsum for epilogue."""
                cs = m_ps.tile([1, BC], fp32, tag="m")
                nc.tensor.matmul(cs, ones_col_bf, v, start=True, stop=True)
                rec = smallp.tile([1, BC], fp32, tag="rec")
                nc.vector.reciprocal(rec, cs)
                k = renorm_k[0]
                renorm_k[0] += 1
                nc.vector.tensor_copy(glog3[:, :, k], cs)
                bc_ps = m_ps.tile([128, BC], fp32, tag="m")
                nc.tensor.matmul(bc_ps, ones_row_f, rec, start=True, stop=True)
                return bc_ps

            def eslice(t):
                return ebuf3[:, :, t]

            vf = eslice(0)          # a_0 = E_0
            vb = eslice(S - 1)      # y_{2047} = E_{2047}
            bc_f = None
            bc_b = None
            vb_fin = None
            NROT = S - 1 - MID      # 1024 rotations
            for r in range(NROT):
                # forward step t = r+1:  a_t = (a_{t-1} @ W) * E_t  (lhsT=W)
                if r < MID:
                    t = r + 1
                    qf = q_ps.tile([128, 512], fp32, tag="q")
                    nc.tensor.matmul(qf[:, 0:BC], w_bf, vf, start=True, stop=True)
                    nvf = statep.tile([128, BC], bf16, tag="vf")
                    nc.vector.tensor_tensor(out=nvf, in0=qf[:, 0:BC], in1=eslice(t), op=ALU.mult)
                    if bc_f is not None:
                        nc.vector.tensor_tensor(out=nvf, in0=nvf, in1=bc_f, op=ALU.mult)
                        bc_f = None
                    vf = nvf
                    if (t % RENORM == 0 or t == 1008) and t < MID:
                        bc_f = renorm(vf)
                # backward: q = W @ y_{t+1}; t from 2046 down to MID
                t = S - 2 - r
                qb = q_ps.tile([128, 512], fp32, tag="q")
                nc.tensor.matmul(qb[:, 0:BC], wt_bf, vb, start=True, stop=True)
                if t == MID:
                    vb_fin = qb  # b_MID = W y_{MID+1}: final, stays in PSUM
                else:
                    nvb = statep.tile([128, BC], bf16, tag="vb")
                    nc.vector.tensor_tensor(out=nvb, in0=qb[:, 0:BC], in1=eslice(t), op=ALU.mult)
                    if bc_b is not None:
                        nc.vector.tensor_tensor(out=nvb, in0=nvb, in1=bc_b, op=ALU.mult)
                        bc_b = None
                    vb = nvb
                    # scale from a renorm at t applies at step t-1; last chance
                    # is t == MID+2
                    if (t % RENORM == 0 or t == 1040) and t > MID + 1:
                        bc_b = renorm(vb)
                if (r + 1) % RENORM == 0 or (r + 2) % RENORM == 0:
                    pass  # keep renorm rotations clean
                elif r % 4 != 3:
                    do_side(1)

            do_side(len(side))

            # ---------------- epilogue ----------------
            # log_Z = ln(sum_j vf*vb_fin) + sum(ln renorm scales) + S*chat
            dotd = dumpp.tile([128, BC], fp32, tag="dotd")
            nc.vector.tensor_tensor(out=dotd, in0=vb_fin[:, 0:BC], in1=vf, op=ALU.mult)
            zs = m_ps.tile([1, BC], fp32, tag="m")
            nc.tensor.matmul(zs, ones_col_f, dotd, start=True, stop=True)
            lnz = smallp.tile([1, BC], fp32, tag="lnz")
            nc.scalar.activation(lnz, zs, AF.Ln)
            lnglog = smallp.tile([1, BC * NRE], fp32, tag="lnglog")
            nc.scalar.activation(lnglog, glog, AF.Ln)
            accsum = smallp.tile([1, BC], fp32, tag="accsum")
            nc.vector.tensor_reduce(
                accsum,
                lnglog.rearrange("p (b k) -> p b k", k=NRE),
                axis=AX.X, op=ALU.add,
            )
            logz = smallp.tile([1, BC], fp32, tag="logz")
            nc.vector.tensor_tensor(out=logz, in0=lnz, in1=accsum, op=ALU.add)
            nc.vector.tensor_scalar(
                out=logz, in0=logz, scalar1=chat_tot, scalar2=None, op0=ALU.add
            )

            # seq score from gsum columns: [2b] = sum(C*trans), [2b+1] = esel
            gs_ps = m_ps.tile([1, 2 * BC], fp32, tag="m")
            nc.tensor.matmul(gs_ps, ones_col_f, gsum, start=True, stop=True)
            res = smallp.tile([1, BC], fp32, tag="res")
            seq = gs_ps.rearrange("p (b c) -> p b c", c=2)
            nc.vector.tensor_tensor(out=res, in0=logz, in1=seq[:, :, 0], op=ALU.subtract)
            nc.vector.tensor_tensor(out=res, in0=res, in1=seq[:, :, 1], op=ALU.subtract)
            nc.sync.dma_start(out=out_d[:], in_=res[0:1, :])

    return nc


def _get_compiled(finalized=False):
    global _compiled
    if _compiled is None:
        try:
            _compiled = _build_program(use_gpsimd=True)
        except Exception:
            _compiled = _build_program(use_gpsimd=False)
    if finalized and not _compiled.is_finalized():
        _compiled.finalize()
    return _compiled


def make_in_maps(emissions, transitions, tags):
    in_maps = []
    for c in range(NCORES):
        sl = slice(c * BC, (c + 1) * BC)
        em = np.ascontiguousarray(emissions[sl], dtype=np.float32)
        in_maps.append({
            "emissions_sh": em,
            "emis_t": np.ascontiguousarray(em.transpose(0, 2, 1)),
            "transitions": np.ascontiguousarray(transitions, dtype=np.float32),
            "tags_sh": np.ascontiguousarray(tags[sl]).astype(np.int32),
        })
    return in_maps


def _run_device(emissions, transitions, tags):
    from concourse.bass_utils import run_bass_kernel_spmd

    nc = _get_compiled(finalized=True)
    res = run_bass_kernel_spmd(
        nc, make_in_maps(emissions, transitions, tags), list(range(NCORES))
    )
    parts = np.concatenate([res.results[c]["loss_parts"] for c in range(NCORES)])
    return np.float32(parts.mean())


def _run_host(emissions, transitions, tags, mask):
    """Slow but fully general fallback (any mask pattern)."""
    e = emissions.astype(np.float64)
    t = transitions.astype(np.float64)

    def lse(x, axis):
        m = x.max(axis=axis, keepdims=True)
        return (m + np.log(np.exp(x - m).sum(axis=axis, keepdims=True))).squeeze(axis)

    score = e[:, 0]
    for s in range(1, e.shape[1]):
        nxt = lse(score[:, :, None] + t[None, :, :] + e[:, s, None, :], axis=1)
        score = np.where(mask[:, s, None], nxt, score)
    log_Z = lse(score, axis=1)
    emit = np.take_along_axis(e, tags[..., None].astype(np.int64), axis=2)[..., 0]
    trans_sc = t[tags[:, :-1].astype(np.int64), tags[:, 1:].astype(np.int64)]
    m = mask[:, 1:].astype(np.float64)
    seq = emit[:, 0] + ((trans_sc + emit[:, 1:]) * m).sum(axis=1)
    return np.float32((log_Z - seq).mean())


def kernel(emissions, transitions, tags, mask):
    emissions = np.asarray(emissions)
    transitions = np.asarray(transitions)
    tags = np.asarray(tags)
    mask = np.asarray(mask)
    if emissions.shape != (B, S, T) or not mask.all():
        return _run_host(emissions, transitions, tags, mask)
    return _run_device(emissions, transitions, tags)

